# revision 9
# baseline (speedup 1.0000x reference)
"""Diagonal SSM (B=4, T=4096, D=1024, N=256) on 8 trn2 NeuronCores.

Sharding: core c handles (batch b = c//2, time-half h = c%2).
Per core:
  - load u shard [T/2, D], transpose on PE (float32r) to get D-on-partitions
  - GEMM1/2 (float32r, full rate): lam_pre^T, Bu^T  [N-part, T-free]
  - sigmoid(+bias) on ACT straight out of PSUM
  - diagonal recurrence via DVE tensor_tensor_scan (local scan L, and
    cumulative-product scan C for the cross-half correction)
  - 1KB AllReduce between half-pairs carries the first half's final state
  - H = L + C * h_in  (h_in masked to 0 on first-half cores)
  - GEMM3 (float32r): y = H^T.T @ Wc^T back to natural [T-part, D-free]
The y += u*Dp term (Dp is a [D] vector) is applied on the host during
unsharding; the device kernel computes y = H @ Wc^T.
"""

import numpy as np

import concourse.bass as bass
import concourse.tile as tile
from concourse import bacc, mybir
from concourse import bass_utils
from concourse.masks import make_identity

F32 = mybir.dt.float32
F32R = mybir.dt.float32r
AOP = mybir.AluOpType
ACT_SIGMOID = mybir.ActivationFunctionType.Sigmoid

# problem dims (full)
B_FULL, T_FULL, D_FULL, N_FULL = 4, 4096, 1024, 256
N_CORES = 8

_module_cache = {}

LAST_RESULTS = None  # BassKernelResults of the most recent run (for test.py)


def build_module(TH, D, N, CH):
    """One-core SPMD program. TH = time steps per core, CH = t-chunk size."""
    key = (TH, D, N, CH)
    if key in _module_cache:
        return _module_cache[key]

    P = 128
    n_tiles = N // P           # N partition tiles
    k_tiles = D // P           # contraction tiles for GEMM1/2
    n_chunks = TH // CH        # t-chunks for the streaming phase
    j_sub = CH // P            # 128-row subtiles per t-chunk
    t_tiles = TH // P          # output row tiles for GEMM3
    DC = min(512, D)           # free-dim chunk for PSUM banks (fp32: 512)
    d_chunks = D // DC

    nc = bacc.Bacc(
        "TRN2",
        target_bir_lowering=False,
        debug=False,
        num_devices=N_CORES,
    )

    u = nc.dram_tensor("u", [TH, D], F32, kind="ExternalInput").ap()
    wl = nc.dram_tensor("wl", [N, D], F32, kind="ExternalInput").ap()
    wb = nc.dram_tensor("wb", [N, D], F32, kind="ExternalInput").ap()
    wc = nc.dram_tensor("wc", [D, N], F32, kind="ExternalInput").ap()
    bl = nc.dram_tensor("bl", [N], F32, kind="ExternalInput").ap()
    m_in = nc.dram_tensor("m_in", [P], F32, kind="ExternalInput").ap()
    m_out = nc.dram_tensor("m_out", [P], F32, kind="ExternalInput").ap()
    y = nc.dram_tensor("y", [TH, D], F32, kind="ExternalOutput").ap()

    with tile.TileContext(nc) as tc:
        with (
            tc.tile_pool(name="const", bufs=1) as const,
            tc.tile_pool(name="wtmp", bufs=1) as wtmp,
            tc.tile_pool(name="unat", bufs=2) as unat_pool,
            tc.tile_pool(name="utp", bufs=2) as ut_pool,
            tc.tile_pool(name="lamp", bufs=3) as lam_pool,
            tc.tile_pool(name="big", bufs=1) as big,
            tc.tile_pool(name="small", bufs=1) as small,
            tc.tile_pool(name="yp", bufs=3) as y_pool,
            tc.tile_pool(name="pst", bufs=2, space="PSUM") as psum_t,
            tc.tile_pool(name="psg", bufs=4, space="PSUM") as psum_g,
            tc.tile_pool(name="psy", bufs=2, space="PSUM") as psum_y,
            tc.tile_pool(name="dram", bufs=1, space="DRAM") as dram,
        ):
            # ---- phase 0: constants / weights -------------------------------
            ident = const.tile([P, P], F32)
            nc.gpsimd.memset(ident, 0.0)
            make_identity(nc, ident.bitcast(F32R), nomemset=True)
            identr = ident.bitcast(F32R)

            bl_sb = const.tile([P, n_tiles], F32)
            nc.sync.dma_start(out=bl_sb, in_=bl.rearrange("(a p) -> p a", p=P))
            m_in_sb = const.tile([P, 1], F32)
            nc.sync.dma_start(out=m_in_sb, in_=m_in[:, None])
            m_out_sb = const.tile([P, 1], F32)
            nc.sync.dma_start(out=m_out_sb, in_=m_out[:, None])

            # W_l^T, W_b^T : [P(d), k_tiles, N]  (lhsT tiles for GEMM1/2)
            wlT = const.tile([P, k_tiles, N], F32R)
            wbT = const.tile([P, k_tiles, N], F32R)
            # Wc^T : [P(n), n_tiles, D] (rhs for GEMM3)
            wcT = const.tile([P, n_tiles, D], F32R)

            for (w_nat_name, w_src, w_dst) in (
                ("wl_nat", wl, wlT),
                ("wb_nat", wb, wbT),
            ):
                w_nat = wtmp.tile([P, n_tiles, D], F32R, name=w_nat_name, tag="wtmp")
                nc.sync.dma_start(
                    out=w_nat, in_=w_src.bitcast(F32R).rearrange("(a p) d -> p a d", p=P)
                )
                for a in range(n_tiles):
                    for k4 in range(0, k_tiles, 4):
                        kn = min(4, k_tiles - k4)
                        pt = psum_t.tile([P, 512], F32, name="ptw", tag="pt")
                        for kk in range(kn):
                            k = k4 + kk
                            nc.tensor.transpose(
                                pt[:, kk * P:(kk + 1) * P].bitcast(F32R),
                                w_nat[:, a, k * P:(k + 1) * P],
                                identr,
                            )
                        # dest: [P, kn, P] block at (k4.., a)
                        nc.vector.tensor_copy(
                            w_dst[:, k4:k4 + kn, a * P:(a + 1) * P],
                            pt[:, : kn * P].rearrange("p (k q) -> p k q", k=kn),
                        )

            wc_nat = wtmp.tile([P, k_tiles, N], F32R, tag="wtmp")
            nc.sync.dma_start(out=wc_nat, in_=wc.bitcast(F32R).rearrange("(a p) n -> p a n", p=P))
            for m in range(n_tiles):
                for a4 in range(0, k_tiles, 4):
                    an = min(4, k_tiles - a4)
                    pt = psum_t.tile([P, 512], F32, name="ptc", tag="pt")
                    for aa in range(an):
                        a = a4 + aa
                        nc.tensor.transpose(
                            pt[:, aa * P:(aa + 1) * P].bitcast(F32R),
                            wc_nat[:, a, m * P:(m + 1) * P],
                            identr,
                        )
                    nc.scalar.copy(
                        wcT[:, m, a4 * P:(a4 + an) * P],
                        pt[:, : an * P],
                    )

            # ---- phase A: stream t-chunks -----------------------------------
            # full-TH tensors with N on partitions
            h_sb = big.tile([P, n_tiles, TH], F32)     # local scan L
            c_sb = big.tile([P, n_tiles, TH], F32)     # cumprod of lam
            hf_sb = big.tile([P, n_tiles, TH], F32R)    # corrected H

            u_r = u.bitcast(F32R).rearrange("(c j p) d -> c p j d", c=n_chunks, p=P)

            for c in range(n_chunks):
                u_nat = unat_pool.tile([P, j_sub, D], F32R, tag="unat")
                nc.sync.dma_start(out=u_nat, in_=u_r[c])

                uT = ut_pool.tile([P, k_tiles, CH], F32R, tag="uT")
                for k in range(k_tiles):
                    pt = psum_t.tile([P, 512], F32, name="ptu", tag="pt")
                    for j in range(j_sub):
                        nc.tensor.transpose(
                            pt[:, j * P:(j + 1) * P].bitcast(F32R),
                            u_nat[:, j, k * P:(k + 1) * P],
                            identr,
                        )
                    if k % 2 == 0:
                        nc.vector.tensor_copy(uT[:, k, :], pt[:, :CH])
                    else:
                        nc.scalar.copy(uT[:, k, :], pt[:, :CH])

                lam_sb = lam_pool.tile([P, n_tiles, CH], F32, tag="lam")
                for n in range(n_tiles):
                    ps_l = psum_g.tile([P, CH], F32, name="psl", tag="psg")
                    ps_b = psum_g.tile([P, CH], F32, name="psb", tag="psg")
                    for k in range(k_tiles):
                        nc.tensor.matmul(
                            ps_l,
                            wlT[:, k, n * P:(n + 1) * P],
                            uT[:, k, :],
                            start=(k == 0),
                            stop=(k == k_tiles - 1),
                        )
                        nc.tensor.matmul(
                            ps_b,
                            wbT[:, k, n * P:(n + 1) * P],
                            uT[:, k, :],
                            start=(k == 0),
                            stop=(k == k_tiles - 1),
                        )
                    # lam = sigmoid(pre + bl)   (PSUM -> SBUF on ACT)
                    nc.scalar.activation(
                        lam_sb[:, n, :], ps_l, ACT_SIGMOID,
                        bias=bl_sb[:, n:n + 1],
                    )
                    cs = slice(c * CH, (c + 1) * CH)
                    # local scan: L_t = lam_t * L_{t-1} + bu_t
                    nc.vector.tensor_tensor_scan(
                        h_sb[:, n, cs], lam_sb[:, n, :], ps_b,
                        0.0 if c == 0 else h_sb[:, n, c * CH - 1:c * CH],
                        AOP.mult, AOP.add,
                    )
                    # cumprod: C_t = lam_t * C_{t-1}
                    nc.vector.tensor_tensor_scan(
                        c_sb[:, n, cs], lam_sb[:, n, :], lam_sb[:, n, :],
                        1.0 if c == 0 else c_sb[:, n, c * CH - 1:c * CH],
                        AOP.mult, AOP.bypass,
                    )

            # ---- phase B: exchange boundary state ---------------------------
            cc_in = dram.tile([P, n_tiles], F32, addr_space="Local")
            cc_out = dram.tile([P, n_tiles], F32, addr_space="Local")
            s_m = small.tile([P, n_tiles, 1], F32)
            # mask: only first-half cores contribute their final state
            nc.vector.tensor_scalar_mul(s_m, h_sb[:, :, TH - 1:TH], m_in_sb)
            nc.sync.dma_start(out=cc_in, in_=s_m[:, :, 0])
            nc.gpsimd.collective_compute(
                "AllReduce",
                AOP.add,
                replica_groups=[[2 * i, 2 * i + 1] for i in range(N_CORES // 2)],
                ins=[cc_in.opt()],
                outs=[cc_out.opt()],
            )
            hin_raw = small.tile([P, n_tiles], F32)
            nc.sync.dma_start(out=hin_raw, in_=cc_out)
            hin = small.tile([P, n_tiles], F32)
            # only second-half cores apply the incoming state
            nc.vector.tensor_scalar_mul(hin, hin_raw, m_out_sb)

            # H = C * h_in + L
            for n in range(n_tiles):
                nc.vector.scalar_tensor_tensor(
                    hf_sb[:, n, :], c_sb[:, n, :], hin[:, n:n + 1], h_sb[:, n, :],
                    AOP.mult, AOP.add,
                )

            # ---- phase C: GEMM3, back to natural layout ---------------------
            y_r = y.rearrange("(tt p) d -> tt p d", p=P)
            for tt in range(t_tiles):
                ps_ys = [
                    psum_y.tile([P, DC], F32, name=f"psy{dcs}", tag="psy")
                    for dcs in range(d_chunks)
                ]
                for n in range(n_tiles):
                    lhsT = hf_sb[:, n, tt * P:(tt + 1) * P]
                    for dc in range(d_chunks):
                        nc.tensor.matmul(
                            ps_ys[dc],
                            lhsT,
                            wcT[:, n, dc * DC:(dc + 1) * DC],
                            start=(n == 0),
                            stop=(n == n_tiles - 1),
                        )
                y_t = y_pool.tile([P, D], F32, tag="yt")
                for dc in range(d_chunks):
                    if dc % 2 == 0:
                        nc.scalar.copy(y_t[:, dc * DC:(dc + 1) * DC], ps_ys[dc])
                    else:
                        nc.vector.tensor_copy(y_t[:, dc * DC:(dc + 1) * DC], ps_ys[dc])
                nc.sync.dma_start(out=y_r[tt], in_=y_t)

    nc.compile()
    _module_cache[key] = nc
    return nc


def make_in_maps(u_full, Wl, bl, Wb, Wc, TH):
    """Per-core input dicts. Core c -> (batch c//2, half c%2)."""
    P = 128
    in_maps = []
    for c in range(N_CORES):
        b, half = c // 2, c % 2
        in_maps.append({
            "u": np.ascontiguousarray(u_full[b, half * TH:(half + 1) * TH, :]),
            "wl": Wl,
            "wb": Wb,
            "wc": Wc,
            "bl": bl,
            "m_in": np.full([P], 1.0 - half, np.float32),
            "m_out": np.full([P], float(half), np.float32),
        })
    return in_maps


def kernel(u, Wl, bl, Wb, Wc, Dp):
    global LAST_RESULTS
    u = np.asarray(u, np.float32)
    Wl = np.ascontiguousarray(np.asarray(Wl, np.float32))
    bl = np.ascontiguousarray(np.asarray(bl, np.float32))
    Wb = np.ascontiguousarray(np.asarray(Wb, np.float32))
    Wc = np.ascontiguousarray(np.asarray(Wc, np.float32))
    Dp = np.asarray(Dp, np.float32)

    B, T, D = u.shape
    N = Wl.shape[0]
    TH = T // 2
    nc = build_module(TH, D, N, 512)
    in_maps = make_in_maps(u, Wl, bl, Wb, Wc, TH)
    res = bass_utils.run_bass_kernel_spmd(
        nc, in_maps, core_ids=list(range(N_CORES))
    )
    LAST_RESULTS = res
    y = np.empty((B, T, D), np.float32)
    for c in range(N_CORES):
        b, half = c // 2, c % 2
        y[b, half * TH:(half + 1) * TH, :] = res.results[c]["y"]
    y += u * Dp[None, None, :]
    return y


# revision 10
# speedup vs baseline: 1.1768x; 1.1768x over previous
"""Diagonal SSM (B=4, T=4096, D=1024, N=256) on 8 trn2 NeuronCores.

Sharding: core c handles (batch b = c//2, time-half h = c%2).
Per core:
  - load u shard [T/2, D], transpose on PE (float32r) to get D-on-partitions
  - GEMM1/2 (float32r, full rate): lam_pre^T, Bu^T  [N-part, T-free]
  - sigmoid(+bias) on ACT straight out of PSUM
  - diagonal recurrence via DVE tensor_tensor_scan (local scan L, and
    cumulative-product scan C for the cross-half correction)
  - 1KB AllReduce between half-pairs carries the first half's final state
  - H = L + C * h_in  (h_in masked to 0 on first-half cores)
  - GEMM3 (float32r): y = H^T.T @ Wc^T back to natural [T-part, D-free]
The y += u*Dp term (Dp is a [D] vector) is applied on the host during
unsharding; the device kernel computes y = H @ Wc^T.
"""

import numpy as np

import concourse.bass as bass
import concourse.tile as tile
from concourse import bacc, mybir
from concourse import bass_utils
from concourse.masks import make_identity

F32 = mybir.dt.float32
F32R = mybir.dt.float32r
AOP = mybir.AluOpType
ACT_SIGMOID = mybir.ActivationFunctionType.Sigmoid

# problem dims (full)
B_FULL, T_FULL, D_FULL, N_FULL = 4, 4096, 1024, 256
N_CORES = 8

_module_cache = {}

LAST_RESULTS = None  # BassKernelResults of the most recent run (for test.py)


def build_module(TH, D, N, CH):
    """One-core SPMD program. TH = time steps per core, CH = t-chunk size."""
    key = (TH, D, N, CH)
    if key in _module_cache:
        return _module_cache[key]

    P = 128
    n_tiles = N // P           # N partition tiles
    k_tiles = D // P           # contraction tiles for GEMM1/2
    n_chunks = TH // CH        # t-chunks for the streaming phase
    j_sub = CH // P            # 128-row subtiles per t-chunk
    t_tiles = TH // P          # output row tiles for GEMM3
    DC = min(512, D)           # free-dim chunk for PSUM banks (fp32: 512)
    d_chunks = D // DC

    nc = bacc.Bacc(
        "TRN2",
        target_bir_lowering=False,
        debug=False,
        num_devices=N_CORES,
    )

    u = nc.dram_tensor("u", [TH, D], F32, kind="ExternalInput").ap()
    wl = nc.dram_tensor("wl", [N, D], F32, kind="ExternalInput").ap()
    wb = nc.dram_tensor("wb", [N, D], F32, kind="ExternalInput").ap()
    wc = nc.dram_tensor("wc", [D, N], F32, kind="ExternalInput").ap()
    bl = nc.dram_tensor("bl", [N], F32, kind="ExternalInput").ap()
    m_in = nc.dram_tensor("m_in", [P], F32, kind="ExternalInput").ap()
    m_out = nc.dram_tensor("m_out", [P], F32, kind="ExternalInput").ap()
    y = nc.dram_tensor("y", [TH, D], F32, kind="ExternalOutput").ap()

    with tile.TileContext(nc) as tc:
        with (
            tc.tile_pool(name="const", bufs=1) as const,
            tc.tile_pool(name="wtmp", bufs=1) as wtmp,
            tc.tile_pool(name="unat", bufs=2) as unat_pool,
            tc.tile_pool(name="utp", bufs=2) as ut_pool,
            tc.tile_pool(name="lamp", bufs=3) as lam_pool,
            tc.tile_pool(name="big", bufs=1) as big,
            tc.tile_pool(name="small", bufs=1) as small,
            tc.tile_pool(name="yp", bufs=3) as y_pool,
            tc.tile_pool(name="pst", bufs=2, space="PSUM") as psum_t,
            tc.tile_pool(name="psg", bufs=4, space="PSUM") as psum_g,
            tc.tile_pool(name="psy", bufs=2, space="PSUM") as psum_y,
            tc.tile_pool(name="dram", bufs=1, space="DRAM") as dram,
        ):
            # ---- phase -1: warm up the collective firmware ------------------
            warm_in = dram.tile([P, 1], F32)
            warm_out = dram.tile([P, 1], F32)
            warm_sb = small.tile([P, 1], F32)
            nc.vector.memset(warm_sb, 0.0)
            nc.sync.dma_start(out=warm_in, in_=warm_sb)
            nc.gpsimd.collective_compute(
                "AllReduce",
                AOP.add,
                replica_groups=[[2 * i, 2 * i + 1] for i in range(N_CORES // 2)],
                ins=[warm_in.opt()],
                outs=[warm_out.opt()],
            )

            # ---- phase 0: constants / weights -------------------------------
            ident = const.tile([P, P], F32)
            nc.gpsimd.memset(ident, 0.0)
            make_identity(nc, ident.bitcast(F32R), nomemset=True)
            identr = ident.bitcast(F32R)

            bl_sb = const.tile([P, n_tiles], F32)
            nc.sync.dma_start(out=bl_sb, in_=bl.rearrange("(a p) -> p a", p=P))
            m_in_sb = const.tile([P, 1], F32)
            nc.sync.dma_start(out=m_in_sb, in_=m_in[:, None])
            m_out_sb = const.tile([P, 1], F32)
            nc.sync.dma_start(out=m_out_sb, in_=m_out[:, None])

            u_r = u.bitcast(F32R).rearrange("(c j p) d -> c p j d", c=n_chunks, p=P)

            def load_u_chunk(c):
                u_nat = unat_pool.tile([P, j_sub, D], F32R, tag="unat", name=f"u_nat{c}")
                nc.sync.dma_start(out=u_nat, in_=u_r[c])
                return u_nat

            def transpose_u_chunk(c, u_nat):
                uT = ut_pool.tile([P, k_tiles, CH], F32R, tag="uT", name=f"uT{c}")
                for k in range(k_tiles):
                    pt = psum_t.tile([P, 512], F32, name="ptu", tag="pt")
                    for j in range(j_sub):
                        nc.tensor.transpose(
                            pt[:, j * P:(j + 1) * P].bitcast(F32R),
                            u_nat[:, j, k * P:(k + 1) * P],
                            identr,
                        )
                    if k % 2 == 0:
                        nc.vector.tensor_copy(uT[:, k, :], pt[:, :CH])
                    else:
                        nc.scalar.copy(uT[:, k, :], pt[:, :CH])
                return uT

            u_nat0 = load_u_chunk(0)
            uT0 = transpose_u_chunk(0, u_nat0)

            # W_l^T, W_b^T : [P(d), k_tiles, N]  (lhsT tiles for GEMM1/2)
            wlT = const.tile([P, k_tiles, N], F32R)
            wbT = const.tile([P, k_tiles, N], F32R)
            # Wc^T : [P(n), n_tiles, D] (rhs for GEMM3)
            wcT = const.tile([P, n_tiles, D], F32R)

            for (w_nat_name, w_src, w_dst) in (
                ("wl_nat", wl, wlT),
                ("wb_nat", wb, wbT),
            ):
                w_nat = wtmp.tile([P, n_tiles, D], F32R, name=w_nat_name, tag="wtmp")
                nc.sync.dma_start(
                    out=w_nat, in_=w_src.bitcast(F32R).rearrange("(a p) d -> p a d", p=P)
                )
                for a in range(n_tiles):
                    for k4 in range(0, k_tiles, 4):
                        kn = min(4, k_tiles - k4)
                        pt = psum_t.tile([P, 512], F32, name="ptw", tag="pt")
                        for kk in range(kn):
                            k = k4 + kk
                            nc.tensor.transpose(
                                pt[:, kk * P:(kk + 1) * P].bitcast(F32R),
                                w_nat[:, a, k * P:(k + 1) * P],
                                identr,
                            )
                        # dest: [P, kn, P] block at (k4.., a)
                        nc.vector.tensor_copy(
                            w_dst[:, k4:k4 + kn, a * P:(a + 1) * P],
                            pt[:, : kn * P].rearrange("p (k q) -> p k q", k=kn),
                        )

            # ---- phase A: stream t-chunks -----------------------------------
            # full-TH tensors with N on partitions
            h_sb = big.tile([P, n_tiles, TH], F32)     # local scan L
            c_sb = big.tile([P, n_tiles, TH], F32)     # cumprod of lam
            hf_sb = big.tile([P, n_tiles, TH], F32R)    # corrected H

            for c in range(n_chunks):
                if c == 0:
                    uT = uT0
                else:
                    u_nat = load_u_chunk(c)
                    uT = transpose_u_chunk(c, u_nat)

                lam_sb = lam_pool.tile([P, n_tiles, CH], F32, tag="lam")
                for n in range(n_tiles):
                    ps_l = psum_g.tile([P, CH], F32, name="psl", tag="psg")
                    ps_b = psum_g.tile([P, CH], F32, name="psb", tag="psg")
                    for k in range(k_tiles):
                        nc.tensor.matmul(
                            ps_l,
                            wlT[:, k, n * P:(n + 1) * P],
                            uT[:, k, :],
                            start=(k == 0),
                            stop=(k == k_tiles - 1),
                        )
                        nc.tensor.matmul(
                            ps_b,
                            wbT[:, k, n * P:(n + 1) * P],
                            uT[:, k, :],
                            start=(k == 0),
                            stop=(k == k_tiles - 1),
                        )
                    # lam = sigmoid(pre + bl)   (PSUM -> SBUF on ACT)
                    nc.scalar.activation(
                        lam_sb[:, n, :], ps_l, ACT_SIGMOID,
                        bias=bl_sb[:, n:n + 1],
                    )
                    cs = slice(c * CH, (c + 1) * CH)
                    # local scan: L_t = lam_t * L_{t-1} + bu_t
                    nc.vector.tensor_tensor_scan(
                        h_sb[:, n, cs], lam_sb[:, n, :], ps_b,
                        0.0 if c == 0 else h_sb[:, n, c * CH - 1:c * CH],
                        AOP.mult, AOP.add,
                    )
                    # cumprod: C_t = lam_t * C_{t-1}
                    nc.vector.tensor_tensor_scan(
                        c_sb[:, n, cs], lam_sb[:, n, :], lam_sb[:, n, :],
                        1.0 if c == 0 else c_sb[:, n, c * CH - 1:c * CH],
                        AOP.mult, AOP.bypass,
                    )

            wc_nat = wtmp.tile([P, k_tiles, N], F32R, tag="wctmp")
            nc.sync.dma_start(out=wc_nat, in_=wc.bitcast(F32R).rearrange("(a p) n -> p a n", p=P))
            for m in range(n_tiles):
                for a4 in range(0, k_tiles, 4):
                    an = min(4, k_tiles - a4)
                    pt = psum_t.tile([P, 512], F32, name="ptc", tag="pt")
                    for aa in range(an):
                        a = a4 + aa
                        nc.tensor.transpose(
                            pt[:, aa * P:(aa + 1) * P].bitcast(F32R),
                            wc_nat[:, a, m * P:(m + 1) * P],
                            identr,
                        )
                    nc.scalar.copy(
                        wcT[:, m, a4 * P:(a4 + an) * P],
                        pt[:, : an * P],
                    )

            # ---- phase B: exchange boundary state ---------------------------
            cc_in = dram.tile([P, n_tiles], F32, addr_space="Local")
            cc_out = dram.tile([P, n_tiles], F32, addr_space="Local")
            s_m = small.tile([P, n_tiles, 1], F32)
            # mask: only first-half cores contribute their final state
            nc.vector.tensor_scalar_mul(s_m, h_sb[:, :, TH - 1:TH], m_in_sb)
            nc.sync.dma_start(out=cc_in, in_=s_m[:, :, 0])
            nc.gpsimd.collective_compute(
                "AllReduce",
                AOP.add,
                replica_groups=[[2 * i, 2 * i + 1] for i in range(N_CORES // 2)],
                ins=[cc_in.opt()],
                outs=[cc_out.opt()],
            )
            hin_raw = small.tile([P, n_tiles], F32)
            nc.sync.dma_start(out=hin_raw, in_=cc_out)
            hin = small.tile([P, n_tiles], F32)
            # only second-half cores apply the incoming state
            nc.vector.tensor_scalar_mul(hin, hin_raw, m_out_sb)

            # H = C * h_in + L
            for n in range(n_tiles):
                nc.vector.scalar_tensor_tensor(
                    hf_sb[:, n, :], c_sb[:, n, :], hin[:, n:n + 1], h_sb[:, n, :],
                    AOP.mult, AOP.add,
                )

            # ---- phase C: GEMM3, back to natural layout ---------------------
            y_r = y.rearrange("(tt p) d -> tt p d", p=P)
            for tt in range(t_tiles):
                pool_c = psum_y if tt % 2 == 0 else psum_t
                ps_ys = [
                    pool_c.tile([P, DC], F32, name=f"psy{dcs}",
                                tag="psy" if tt % 2 == 0 else "pt")
                    for dcs in range(d_chunks)
                ]
                for n in range(n_tiles):
                    lhsT = hf_sb[:, n, tt * P:(tt + 1) * P]
                    for dc in range(d_chunks):
                        nc.tensor.matmul(
                            ps_ys[dc],
                            lhsT,
                            wcT[:, n, dc * DC:(dc + 1) * DC],
                            start=(n == 0),
                            stop=(n == n_tiles - 1),
                        )
                y_t = y_pool.tile([P, D], F32, tag="yt")
                for dc in range(d_chunks):
                    if dc % 2 == 0:
                        nc.scalar.copy(y_t[:, dc * DC:(dc + 1) * DC], ps_ys[dc])
                    else:
                        nc.vector.tensor_copy(y_t[:, dc * DC:(dc + 1) * DC], ps_ys[dc])
                nc.sync.dma_start(out=y_r[tt], in_=y_t)

    nc.compile()
    _module_cache[key] = nc
    return nc


def make_in_maps(u_full, Wl, bl, Wb, Wc, TH):
    """Per-core input dicts. Core c -> (batch c//2, half c%2)."""
    P = 128
    in_maps = []
    for c in range(N_CORES):
        b, half = c // 2, c % 2
        in_maps.append({
            "u": np.ascontiguousarray(u_full[b, half * TH:(half + 1) * TH, :]),
            "wl": Wl,
            "wb": Wb,
            "wc": Wc,
            "bl": bl,
            "m_in": np.full([P], 1.0 - half, np.float32),
            "m_out": np.full([P], float(half), np.float32),
        })
    return in_maps


def kernel(u, Wl, bl, Wb, Wc, Dp):
    global LAST_RESULTS
    u = np.asarray(u, np.float32)
    Wl = np.ascontiguousarray(np.asarray(Wl, np.float32))
    bl = np.ascontiguousarray(np.asarray(bl, np.float32))
    Wb = np.ascontiguousarray(np.asarray(Wb, np.float32))
    Wc = np.ascontiguousarray(np.asarray(Wc, np.float32))
    Dp = np.asarray(Dp, np.float32)

    B, T, D = u.shape
    N = Wl.shape[0]
    TH = T // 2
    nc = build_module(TH, D, N, 512)
    in_maps = make_in_maps(u, Wl, bl, Wb, Wc, TH)
    res = bass_utils.run_bass_kernel_spmd(
        nc, in_maps, core_ids=list(range(N_CORES))
    )
    LAST_RESULTS = res
    y = np.empty((B, T, D), np.float32)
    for c in range(N_CORES):
        b, half = c // 2, c % 2
        y[b, half * TH:(half + 1) * TH, :] = res.results[c]["y"]
    y += u * Dp[None, None, :]
    return y


# revision 15
# speedup vs baseline: 1.2085x; 1.0269x over previous
"""Diagonal SSM (B=4, T=4096, D=1024, N=256) on 8 trn2 NeuronCores.

Sharding: core c handles (batch b = c//2, time-half h = c%2).
Per core:
  - load u shard [T/2, D], transpose on PE (float32r) to get D-on-partitions
  - GEMM1/2 (float32r, full rate): lam_pre^T, Bu^T  [N-part, T-free]
  - sigmoid(+bias) on ACT straight out of PSUM
  - diagonal recurrence via DVE tensor_tensor_scan: local scan L (zero init)
    and cumprod scan C of lam
  - 1KB AllReduce between half-pairs carries the first half's final state
  - GEMM3 split into two passes so the collective latency is hidden:
      L-pass:  yL = L^T.T @ Wc^T          (independent of the collective)
      C-pass:  y  = yL + C^T.T @ (h_in * Wc^T)
    (h_in masked to 0 on first-half cores, so y == yL there numerically)
The y += u*Dp term (Dp is a [D] vector) is applied on the host during
unsharding; the device kernel computes y = H @ Wc^T.
"""

import numpy as np

import concourse.bass as bass
import concourse.tile as tile
from concourse import bacc, mybir
from concourse import bass_utils
from concourse.masks import make_identity

F32 = mybir.dt.float32
F32R = mybir.dt.float32r
AOP = mybir.AluOpType
ACT_SIGMOID = mybir.ActivationFunctionType.Sigmoid
ACT_COPY = mybir.ActivationFunctionType.Copy

# problem dims (full)
B_FULL, T_FULL, D_FULL, N_FULL = 4, 4096, 1024, 256
N_CORES = 8

_module_cache = {}

LAST_RESULTS = None  # BassKernelResults of the most recent run (for test.py)


def build_module(TH, D, N, CH):
    """One-core SPMD program. TH = time steps per core, CH = t-chunk size."""
    key = (TH, D, N, CH)
    if key in _module_cache:
        return _module_cache[key]

    P = 128
    n_tiles = N // P           # N partition tiles
    k_tiles = D // P           # contraction tiles for GEMM1/2
    n_chunks = TH // CH        # t-chunks for the streaming phase
    j_sub = CH // P            # 128-row subtiles per t-chunk
    t_tiles = TH // P          # output row tiles for GEMM3
    DC = min(512, D)           # free-dim chunk for PSUM banks (fp32: 512)
    d_chunks = D // DC

    nc = bacc.Bacc(
        "TRN2",
        target_bir_lowering=False,
        debug=False,
        num_devices=N_CORES,
    )

    u = nc.dram_tensor("u", [TH, D], F32, kind="ExternalInput").ap()
    wl = nc.dram_tensor("wl", [N, D], F32, kind="ExternalInput").ap()
    wb = nc.dram_tensor("wb", [N, D], F32, kind="ExternalInput").ap()
    wc = nc.dram_tensor("wc", [D, N], F32, kind="ExternalInput").ap()
    bl = nc.dram_tensor("bl", [N], F32, kind="ExternalInput").ap()
    m_in = nc.dram_tensor("m_in", [P], F32, kind="ExternalInput").ap()
    m_out = nc.dram_tensor("m_out", [P], F32, kind="ExternalInput").ap()
    y = nc.dram_tensor("y", [TH, D], F32, kind="ExternalOutput").ap()

    RG = [[2 * i, 2 * i + 1] for i in range(N_CORES // 2)]

    with tile.TileContext(nc) as tc:
        with (
            tc.tile_pool(name="const", bufs=1) as const,
            tc.tile_pool(name="wtmp", bufs=1) as wtmp,
            tc.tile_pool(name="unat", bufs=2) as unat_pool,
            tc.tile_pool(name="utp", bufs=2) as ut_pool,
            tc.tile_pool(name="lamp", bufs=2) as lam_pool,
            tc.tile_pool(name="big", bufs=1) as big,
            tc.tile_pool(name="small", bufs=1) as small,
            tc.tile_pool(name="ylp", bufs=8) as yl_pool,
            tc.tile_pool(name="yp", bufs=3) as y_pool,
            tc.tile_pool(name="pst", bufs=3, space="PSUM") as psum_t,
            tc.tile_pool(name="psg", bufs=4, space="PSUM") as psum_g,
            tc.tile_pool(name="psy", bufs=1, space="PSUM") as psum_y,
            tc.tile_pool(name="dram", bufs=1, space="DRAM") as dram,
        ):
            # ---- phase -1: warm up the collective firmware ------------------
            warm_in = dram.tile([P, 1], F32)
            warm_out = dram.tile([P, 1], F32)
            warm_sb = small.tile([P, 1], F32)
            nc.vector.memset(warm_sb, 0.0)
            nc.sync.dma_start(out=warm_in, in_=warm_sb)
            nc.gpsimd.collective_compute(
                "AllReduce", AOP.add, replica_groups=RG,
                ins=[warm_in.opt()], outs=[warm_out.opt()],
            )

            # ---- phase 0: constants -----------------------------------------
            ident = const.tile([P, P], F32)
            nc.gpsimd.memset(ident, 0.0)
            make_identity(nc, ident.bitcast(F32R), nomemset=True)
            identr = ident.bitcast(F32R)

            bl_sb = const.tile([P, n_tiles], F32)
            nc.sync.dma_start(out=bl_sb, in_=bl.rearrange("(a p) -> p a", p=P))
            m_in_sb = const.tile([P, 1], F32)
            nc.sync.dma_start(out=m_in_sb, in_=m_in[:, None])
            m_out_sb = const.tile([P, 1], F32)
            nc.sync.dma_start(out=m_out_sb, in_=m_out[:, None])

            u_r = u.bitcast(F32R).rearrange(
                "(c j p) d -> c j p d", c=n_chunks, p=P
            )

            # chunk 0: load per j-subtile so transposes start on first arrival
            u_nat0 = unat_pool.tile([P, j_sub, D], F32R, tag="unat", name="u_nat0")
            for j in range(j_sub):
                nc.sync.dma_start(out=u_nat0[:, j, :], in_=u_r[0, j])

            uT0 = ut_pool.tile([P, k_tiles, CH], F32R, tag="uT", name="uT0")
            for j in range(j_sub):
                for k4 in range(0, k_tiles, 4):
                    kn = min(4, k_tiles - k4)
                    pt = psum_t.tile([P, 512], F32, name="ptu0", tag="pt")
                    for kk in range(kn):
                        k = k4 + kk
                        nc.tensor.transpose(
                            pt[:, kk * P:(kk + 1) * P].bitcast(F32R),
                            u_nat0[:, j, k * P:(k + 1) * P],
                            identr,
                        )
                    dst = uT0[:, k4:k4 + kn, j * P:(j + 1) * P]
                    srcv = pt[:, : kn * P].rearrange("p (k q) -> p k q", k=kn)
                    if (j + k4) % 2 == 0:
                        nc.vector.tensor_copy(dst, srcv)
                    else:
                        nc.scalar.copy(dst, srcv)

            # W_l^T, W_b^T : [P(d), k_tiles, N]  (lhsT tiles for GEMM1/2)
            wlT = const.tile([P, k_tiles, N], F32R)
            wbT = const.tile([P, k_tiles, N], F32R)
            # Wc^T and its h_in-scaled copy : [P(n), n_tiles, D]
            wcT = const.tile([P, n_tiles, D], F32R)
            wcs = const.tile([P, n_tiles, D], F32R)

            def transpose_w(w_src, w_dst, nm):
                w_nat = wtmp.tile([P, n_tiles, D], F32R, name=nm, tag="wtmp")
                nc.sync.dma_start(
                    out=w_nat,
                    in_=w_src.bitcast(F32R).rearrange("(a p) d -> p a d", p=P),
                )
                for a in range(n_tiles):
                    for k4 in range(0, k_tiles, 4):
                        kn = min(4, k_tiles - k4)
                        pt = psum_t.tile([P, 512], F32, name="ptw", tag="pt")
                        for kk in range(kn):
                            k = k4 + kk
                            nc.tensor.transpose(
                                pt[:, kk * P:(kk + 1) * P].bitcast(F32R),
                                w_nat[:, a, k * P:(k + 1) * P],
                                identr,
                            )
                        dst = w_dst[:, k4:k4 + kn, a * P:(a + 1) * P]
                        srcv = pt[:, : kn * P].rearrange("p (k q) -> p k q", k=kn)
                        if (a + k4 // 4) % 2 == 0:
                            nc.vector.tensor_copy(dst, srcv)
                        else:
                            nc.scalar.copy(dst, srcv)

            # ---- phase A: stream t-chunks -----------------------------------
            # full-TH scan outputs with N on partitions (f32r: feed GEMM3)
            h_sb = big.tile([P, n_tiles, TH], F32R)    # local scan L
            c_sb = big.tile([P, n_tiles, TH], F32R)    # cumprod of lam

            def gemm12(uT, wT, psum_tag):
                """One of the two input GEMMs for a chunk -> psum tiles per n."""
                outs = []
                for n in range(n_tiles):
                    ps = psum_g.tile([P, CH], F32, name=psum_tag, tag="psg")
                    for k in range(k_tiles):
                        nc.tensor.matmul(
                            ps,
                            wT[:, k, n * P:(n + 1) * P],
                            uT[:, k, :],
                            start=(k == 0),
                            stop=(k == k_tiles - 1),
                        )
                    outs.append(ps)
                return outs

            def sigmoid_scans(c, ps_ls, ps_bs):
                lam_sb = lam_pool.tile([P, n_tiles, CH], F32, tag="lam",
                                       name=f"lam{c}")
                cs = slice(c * CH, (c + 1) * CH)
                for n in range(n_tiles):
                    nc.scalar.activation(
                        lam_sb[:, n, :], ps_ls[n], ACT_SIGMOID,
                        bias=bl_sb[:, n:n + 1],
                    )
                    # local scan: L_t = lam_t * L_{t-1} + bu_t
                    nc.vector.tensor_tensor_scan(
                        h_sb[:, n, cs], lam_sb[:, n, :], ps_bs[n],
                        0.0 if c == 0 else h_sb[:, n, c * CH - 1:c * CH],
                        AOP.mult, AOP.add,
                    )
                    # cumprod: C_t = lam_t * C_{t-1}
                    nc.vector.tensor_tensor_scan(
                        c_sb[:, n, cs], lam_sb[:, n, :], lam_sb[:, n, :],
                        1.0 if c == 0 else c_sb[:, n, c * CH - 1:c * CH],
                        AOP.mult, AOP.bypass,
                    )

            # chunk 0 interleaved with weight transposes so PE never stalls:
            transpose_w(wl, wlT, "wl_nat")
            ps_ls0 = gemm12(uT0, wlT, "psl")
            transpose_w(wb, wbT, "wb_nat")
            ps_bs0 = gemm12(uT0, wbT, "psb")
            sigmoid_scans(0, ps_ls0, ps_bs0)

            for c in range(1, n_chunks):
                u_nat = unat_pool.tile([P, j_sub, D], F32R, tag="unat",
                                       name=f"u_nat{c}")
                nc.sync.dma_start(
                    out=u_nat,
                    in_=u_r[c].rearrange("j p d -> p j d"),
                )
                uT = ut_pool.tile([P, k_tiles, CH], F32R, tag="uT", name=f"uT{c}")
                for k in range(k_tiles):
                    pt = psum_t.tile([P, 512], F32, name="ptu", tag="pt")
                    for j in range(j_sub):
                        nc.tensor.transpose(
                            pt[:, j * P:(j + 1) * P].bitcast(F32R),
                            u_nat[:, j, k * P:(k + 1) * P],
                            identr,
                        )
                    if k % 2 == 0:
                        nc.vector.tensor_copy(uT[:, k, :], pt[:, :CH])
                    else:
                        nc.scalar.copy(uT[:, k, :], pt[:, :CH])

                ps_ls = gemm12(uT, wlT, "psl")
                ps_bs = gemm12(uT, wbT, "psb")
                sigmoid_scans(c, ps_ls, ps_bs)

            # Wc transpose (needed only for phase C)
            wc_nat = wtmp.tile([P, k_tiles, N], F32R, tag="wtmp", name="wc_nat")
            nc.sync.dma_start(
                out=wc_nat,
                in_=wc.bitcast(F32R).rearrange("(a p) n -> p a n", p=P),
            )
            for m in range(n_tiles):
                for a4 in range(0, k_tiles, 4):
                    an = min(4, k_tiles - a4)
                    pt = psum_t.tile([P, 512], F32, name="ptc", tag="pt")
                    for aa in range(an):
                        a = a4 + aa
                        nc.tensor.transpose(
                            pt[:, aa * P:(aa + 1) * P].bitcast(F32R),
                            wc_nat[:, a, m * P:(m + 1) * P],
                            identr,
                        )
                    if (m + a4 // 4) % 2 == 0:
                        nc.scalar.copy(wcT[:, m, a4 * P:(a4 + an) * P],
                                       pt[:, : an * P])
                    else:
                        nc.vector.tensor_copy(wcT[:, m, a4 * P:(a4 + an) * P],
                                              pt[:, : an * P])

            # ---- phase B: exchange boundary state ---------------------------
            cc_in = dram.tile([P, n_tiles], F32, addr_space="Local")
            cc_out = dram.tile([P, n_tiles], F32, addr_space="Local")
            s_m = small.tile([P, n_tiles, 1], F32)
            # mask: only first-half cores contribute their final state
            nc.vector.tensor_scalar_mul(s_m, h_sb[:, :, TH - 1:TH], m_in_sb)
            nc.sync.dma_start(out=cc_in, in_=s_m[:, :, 0])
            nc.gpsimd.collective_compute(
                "AllReduce", AOP.add, replica_groups=RG,
                ins=[cc_in.opt()], outs=[cc_out.opt()],
            )
            hin_raw = small.tile([P, n_tiles], F32)
            nc.sync.dma_start(out=hin_raw, in_=cc_out)
            hin = small.tile([P, n_tiles], F32)
            # only second-half cores apply the incoming state
            nc.vector.tensor_scalar_mul(hin, hin_raw, m_out_sb)
            # wcs = h_in * Wc^T  (on ACT: copy with per-partition scale)
            for n in range(n_tiles):
                nc.scalar.activation(
                    wcs[:, n, :], wcT[:, n, :], ACT_COPY,
                    scale=hin[:, n:n + 1],
                )

            # ---- phase C: GEMM3 in two passes -------------------------------
            y_r = y.rearrange("(tt p) d -> tt p d", p=P)

            def gemm3(tt, lhs_big, rhs, tag):
                # spread the per-tile banks across the two free psum pools
                ps_ys = [
                    (psum_y if dcs % 2 == 0 else psum_t).tile(
                        [P, DC], F32, name=f"{tag}{dcs}",
                        tag="psy" if dcs % 2 == 0 else "pt",
                    )
                    for dcs in range(d_chunks)
                ]
                for n in range(n_tiles):
                    lhsT = lhs_big[:, n, tt * P:(tt + 1) * P]
                    for dc in range(d_chunks):
                        nc.tensor.matmul(
                            ps_ys[dc], lhsT, rhs[:, n, dc * DC:(dc + 1) * DC],
                            start=(n == 0), stop=(n == n_tiles - 1),
                        )
                return ps_ys

            # L-pass runs while the collective is in flight; C-pass consumes
            # the collective result. Interleave emission (L0..L7, then
            # C0,L8,C1,L9,... then C8..C15) so the 8-slot yl pool never
            # creates a cyclic engine-order wait.
            yl_tiles = {}
            n_ahead = min(8, t_tiles)

            def l_pass(tt):
                ps_ys = gemm3(tt, h_sb, wcT, "pl")
                yl = yl_pool.tile([P, D], F32, tag="yl", name=f"yl{tt}")
                for dc in range(d_chunks):
                    if dc % 2 == 0:
                        nc.scalar.copy(yl[:, dc * DC:(dc + 1) * DC], ps_ys[dc])
                    else:
                        nc.vector.tensor_copy(yl[:, dc * DC:(dc + 1) * DC],
                                              ps_ys[dc])
                yl_tiles[tt] = yl

            def c_pass(tt):
                ps_ys = gemm3(tt, c_sb, wcs, "pc")
                y_t = y_pool.tile([P, D], F32, tag="yt", name=f"yt{tt}")
                for dc in range(d_chunks):
                    nc.vector.tensor_add(
                        y_t[:, dc * DC:(dc + 1) * DC],
                        ps_ys[dc],
                        yl_tiles[tt][:, dc * DC:(dc + 1) * DC],
                    )
                nc.sync.dma_start(out=y_r[tt], in_=y_t)

            for tt in range(n_ahead):
                l_pass(tt)
            for tt in range(t_tiles):
                c_pass(tt)
                if tt + n_ahead < t_tiles:
                    l_pass(tt + n_ahead)

    nc.compile()
    _module_cache[key] = nc
    return nc


def make_in_maps(u_full, Wl, bl, Wb, Wc, TH):
    """Per-core input dicts. Core c -> (batch c//2, half c%2)."""
    P = 128
    in_maps = []
    for c in range(N_CORES):
        b, half = c // 2, c % 2
        in_maps.append({
            "u": np.ascontiguousarray(u_full[b, half * TH:(half + 1) * TH, :]),
            "wl": Wl,
            "wb": Wb,
            "wc": Wc,
            "bl": bl,
            "m_in": np.full([P], 1.0 - half, np.float32),
            "m_out": np.full([P], float(half), np.float32),
        })
    return in_maps


def kernel(u, Wl, bl, Wb, Wc, Dp):
    global LAST_RESULTS
    u = np.asarray(u, np.float32)
    Wl = np.ascontiguousarray(np.asarray(Wl, np.float32))
    bl = np.ascontiguousarray(np.asarray(bl, np.float32))
    Wb = np.ascontiguousarray(np.asarray(Wb, np.float32))
    Wc = np.ascontiguousarray(np.asarray(Wc, np.float32))
    Dp = np.asarray(Dp, np.float32)

    B, T, D = u.shape
    N = Wl.shape[0]
    TH = T // 2
    nc = build_module(TH, D, N, 512)
    in_maps = make_in_maps(u, Wl, bl, Wb, Wc, TH)
    res = bass_utils.run_bass_kernel_spmd(
        nc, in_maps, core_ids=list(range(N_CORES))
    )
    LAST_RESULTS = res
    y = np.empty((B, T, D), np.float32)
    for c in range(N_CORES):
        b, half = c // 2, c % 2
        y[b, half * TH:(half + 1) * TH, :] = res.results[c]["y"]
    y += u * Dp[None, None, :]
    return y


# revision 16
# speedup vs baseline: 1.3069x; 1.0815x over previous
"""Diagonal SSM (B=4, T=4096, D=1024, N=256) on 8 trn2 NeuronCores.

Sharding: core c handles (batch b = c//2, time-half h = c%2).
Per core:
  - load u shard [T/2, D], transpose on PE (float32r) to get D-on-partitions
  - GEMM1/2 (float32r, full rate): lam_pre^T, Bu^T  [N-part, T-free]
  - sigmoid(+bias) on ACT straight out of PSUM
  - diagonal recurrence via DVE tensor_tensor_scan: local scan L (zero init)
    and cumprod scan C of lam
  - 1KB AllReduce between half-pairs carries the first half's final state
  - H = L + C * h_in  (h_in masked to 0 on first-half cores)
  - GEMM3 (float32r): y = H^T.T @ Wc^T back to natural [T-part, D-free]
The y += u*Dp term (Dp is a [D] vector) is applied on the host during
unsharding; the device kernel computes y = H @ Wc^T.
"""

import numpy as np

import concourse.bass as bass
import concourse.tile as tile
from concourse import bacc, mybir
from concourse import bass_utils
from concourse.masks import make_identity

F32 = mybir.dt.float32
F32R = mybir.dt.float32r
AOP = mybir.AluOpType
ACT_SIGMOID = mybir.ActivationFunctionType.Sigmoid

# problem dims (full)
B_FULL, T_FULL, D_FULL, N_FULL = 4, 4096, 1024, 256
N_CORES = 8

_module_cache = {}

LAST_RESULTS = None  # BassKernelResults of the most recent run (for test.py)


def build_module(TH, D, N, CH):
    """One-core SPMD program. TH = time steps per core, CH = t-chunk size."""
    key = (TH, D, N, CH)
    if key in _module_cache:
        return _module_cache[key]

    P = 128
    n_tiles = N // P           # N partition tiles
    k_tiles = D // P           # contraction tiles for GEMM1/2
    n_chunks = TH // CH        # t-chunks for the streaming phase
    j_sub = CH // P            # 128-row subtiles per t-chunk
    t_tiles = TH // P          # output row tiles for GEMM3
    DC = min(512, D)           # free-dim chunk for PSUM banks (fp32: 512)
    d_chunks = D // DC

    nc = bacc.Bacc(
        "TRN2",
        target_bir_lowering=False,
        debug=False,
        num_devices=N_CORES,
    )

    u = nc.dram_tensor("u", [TH, D], F32, kind="ExternalInput").ap()
    wl = nc.dram_tensor("wl", [N, D], F32, kind="ExternalInput").ap()
    wb = nc.dram_tensor("wb", [N, D], F32, kind="ExternalInput").ap()
    wc = nc.dram_tensor("wc", [D, N], F32, kind="ExternalInput").ap()
    bl = nc.dram_tensor("bl", [N], F32, kind="ExternalInput").ap()
    m_in = nc.dram_tensor("m_in", [P], F32, kind="ExternalInput").ap()
    m_out = nc.dram_tensor("m_out", [P], F32, kind="ExternalInput").ap()
    y = nc.dram_tensor("y", [TH, D], F32, kind="ExternalOutput").ap()

    RG = [[2 * i, 2 * i + 1] for i in range(N_CORES // 2)]

    with tile.TileContext(nc) as tc:
        with (
            tc.tile_pool(name="const", bufs=1) as const,
            tc.tile_pool(name="wtmp", bufs=1) as wtmp,
            tc.tile_pool(name="unat", bufs=2) as unat_pool,
            tc.tile_pool(name="utp", bufs=2) as ut_pool,
            tc.tile_pool(name="lamp", bufs=2) as lam_pool,
            tc.tile_pool(name="big", bufs=1) as big,
            tc.tile_pool(name="small", bufs=1) as small,
            tc.tile_pool(name="yp", bufs=3) as y_pool,
            tc.tile_pool(name="pst", bufs=4, space="PSUM") as psum_t,
            tc.tile_pool(name="psg", bufs=4, space="PSUM") as psum_g,
            tc.tile_pool(name="dram", bufs=1, space="DRAM") as dram,
        ):
            # ---- phase -1: warm up the collective firmware ------------------
            warm_in = dram.tile([P, 1], F32)
            warm_out = dram.tile([P, 1], F32)
            warm_sb = small.tile([P, 1], F32)
            nc.vector.memset(warm_sb, 0.0)
            nc.sync.dma_start(out=warm_in, in_=warm_sb)
            nc.gpsimd.collective_compute(
                "AllReduce", AOP.add, replica_groups=RG,
                ins=[warm_in.opt()], outs=[warm_out.opt()],
            )

            # ---- phase 0: constants -----------------------------------------
            # u loads go on the Sync HWDGE ring; weights/bias/masks go on the
            # ACT HWDGE ring so a blocked weight DMA never stalls u prefetch.
            ident = const.tile([P, P], F32)
            nc.gpsimd.memset(ident, 0.0)
            make_identity(nc, ident.bitcast(F32R), nomemset=True)
            identr = ident.bitcast(F32R)

            bl_sb = const.tile([P, n_tiles], F32)
            nc.scalar.dma_start(out=bl_sb, in_=bl.rearrange("(a p) -> p a", p=P))
            m_in_sb = const.tile([P, 1], F32)
            nc.scalar.dma_start(out=m_in_sb, in_=m_in[:, None])
            m_out_sb = const.tile([P, 1], F32)
            nc.scalar.dma_start(out=m_out_sb, in_=m_out[:, None])

            u_r = u.bitcast(F32R).rearrange(
                "(c j p) d -> c j p d", c=n_chunks, p=P
            )

            # chunk 0: load per j-subtile so transposes start on first arrival
            u_nat0 = unat_pool.tile([P, j_sub, D], F32R, tag="unat", name="u_nat0")
            for j in range(j_sub):
                nc.sync.dma_start(out=u_nat0[:, j, :], in_=u_r[0, j])

            # both input-side weights in one tile: no pool-slot serialization
            wlb_nat = wtmp.tile([P, 2 * n_tiles, D], F32R, name="wlb_nat",
                                tag="wlb")
            nc.scalar.dma_start(
                out=wlb_nat[:, :n_tiles, :],
                in_=wl.bitcast(F32R).rearrange("(a p) d -> p a d", p=P),
            )
            nc.scalar.dma_start(
                out=wlb_nat[:, n_tiles:, :],
                in_=wb.bitcast(F32R).rearrange("(a p) d -> p a d", p=P),
            )

            uT0 = ut_pool.tile([P, k_tiles, CH], F32R, tag="uT", name="uT0")
            for j in range(j_sub):
                for k4 in range(0, k_tiles, 4):
                    kn = min(4, k_tiles - k4)
                    pt = psum_t.tile([P, 512], F32, name="ptu0", tag="pt")
                    for kk in range(kn):
                        k = k4 + kk
                        nc.tensor.transpose(
                            pt[:, kk * P:(kk + 1) * P].bitcast(F32R),
                            u_nat0[:, j, k * P:(k + 1) * P],
                            identr,
                        )
                    dst = uT0[:, k4:k4 + kn, j * P:(j + 1) * P]
                    srcv = pt[:, : kn * P].rearrange("p (k q) -> p k q", k=kn)
                    if (j + k4) % 2 == 0:
                        nc.vector.tensor_copy(dst, srcv)
                    else:
                        nc.scalar.copy(dst, srcv)

            # W_l^T, W_b^T : [P(d), k_tiles, N]  (lhsT tiles for GEMM1/2)
            wlT = const.tile([P, k_tiles, N], F32R)
            wbT = const.tile([P, k_tiles, N], F32R)
            # Wc^T : [P(n), n_tiles, D] (rhs for GEMM3)
            wcT = const.tile([P, n_tiles, D], F32R)

            def transpose_w(src_off, w_dst):
                for a in range(n_tiles):
                    for k4 in range(0, k_tiles, 4):
                        kn = min(4, k_tiles - k4)
                        pt = psum_t.tile([P, 512], F32, name="ptw", tag="pt")
                        for kk in range(kn):
                            k = k4 + kk
                            nc.tensor.transpose(
                                pt[:, kk * P:(kk + 1) * P].bitcast(F32R),
                                wlb_nat[:, src_off + a, k * P:(k + 1) * P],
                                identr,
                            )
                        dst = w_dst[:, k4:k4 + kn, a * P:(a + 1) * P]
                        srcv = pt[:, : kn * P].rearrange("p (k q) -> p k q", k=kn)
                        if (a + k4 // 4) % 2 == 0:
                            nc.vector.tensor_copy(dst, srcv)
                        else:
                            nc.scalar.copy(dst, srcv)

            # ---- phase A: stream t-chunks -----------------------------------
            # full-TH scan outputs with N on partitions
            h_sb = big.tile([P, n_tiles, TH], F32)     # local scan L
            c_sb = big.tile([P, n_tiles, TH], F32)     # cumprod of lam
            hf_sb = big.tile([P, n_tiles, TH], F32R)   # corrected H

            def gemm12(uT, wT, psum_tag):
                outs = []
                for n in range(n_tiles):
                    ps = psum_g.tile([P, CH], F32, name=psum_tag, tag="psg")
                    for k in range(k_tiles):
                        nc.tensor.matmul(
                            ps,
                            wT[:, k, n * P:(n + 1) * P],
                            uT[:, k, :],
                            start=(k == 0),
                            stop=(k == k_tiles - 1),
                        )
                    outs.append(ps)
                return outs

            def sigmoid_scans(c, ps_ls, ps_bs):
                lam_sb = lam_pool.tile([P, n_tiles, CH], F32, tag="lam",
                                       name=f"lam{c}")
                cs = slice(c * CH, (c + 1) * CH)
                for n in range(n_tiles):
                    nc.scalar.activation(
                        lam_sb[:, n, :], ps_ls[n], ACT_SIGMOID,
                        bias=bl_sb[:, n:n + 1],
                    )
                    # local scan: L_t = lam_t * L_{t-1} + bu_t
                    nc.vector.tensor_tensor_scan(
                        h_sb[:, n, cs], lam_sb[:, n, :], ps_bs[n],
                        0.0 if c == 0 else h_sb[:, n, c * CH - 1:c * CH],
                        AOP.mult, AOP.add,
                    )
                    # cumprod: C_t = lam_t * C_{t-1}
                    nc.vector.tensor_tensor_scan(
                        c_sb[:, n, cs], lam_sb[:, n, :], lam_sb[:, n, :],
                        1.0 if c == 0 else c_sb[:, n, c * CH - 1:c * CH],
                        AOP.mult, AOP.bypass,
                    )

            # chunk 0 interleaved with weight transposes so PE never stalls
            transpose_w(0, wlT)
            ps_ls0 = gemm12(uT0, wlT, "psl")
            transpose_w(n_tiles, wbT)
            ps_bs0 = gemm12(uT0, wbT, "psb")
            sigmoid_scans(0, ps_ls0, ps_bs0)

            for c in range(1, n_chunks):
                u_nat = unat_pool.tile([P, j_sub, D], F32R, tag="unat",
                                       name=f"u_nat{c}")
                nc.sync.dma_start(
                    out=u_nat, in_=u_r[c].rearrange("j p d -> p j d")
                )
                uT = ut_pool.tile([P, k_tiles, CH], F32R, tag="uT", name=f"uT{c}")
                for k in range(k_tiles):
                    pt = psum_t.tile([P, 512], F32, name="ptu", tag="pt")
                    for j in range(j_sub):
                        nc.tensor.transpose(
                            pt[:, j * P:(j + 1) * P].bitcast(F32R),
                            u_nat[:, j, k * P:(k + 1) * P],
                            identr,
                        )
                    if k % 2 == 0:
                        nc.vector.tensor_copy(uT[:, k, :], pt[:, :CH])
                    else:
                        nc.scalar.copy(uT[:, k, :], pt[:, :CH])

                ps_ls = gemm12(uT, wlT, "psl")
                ps_bs = gemm12(uT, wbT, "psb")
                sigmoid_scans(c, ps_ls, ps_bs)

            # Wc transpose (needed only for phase C)
            wc_nat = wtmp.tile([P, k_tiles, N], F32R, tag="wc", name="wc_nat")
            nc.scalar.dma_start(
                out=wc_nat,
                in_=wc.bitcast(F32R).rearrange("(a p) n -> p a n", p=P),
            )
            for m in range(n_tiles):
                for a4 in range(0, k_tiles, 4):
                    an = min(4, k_tiles - a4)
                    pt = psum_t.tile([P, 512], F32, name="ptc", tag="pt")
                    for aa in range(an):
                        a = a4 + aa
                        nc.tensor.transpose(
                            pt[:, aa * P:(aa + 1) * P].bitcast(F32R),
                            wc_nat[:, a, m * P:(m + 1) * P],
                            identr,
                        )
                    if (m + a4 // 4) % 2 == 0:
                        nc.scalar.copy(wcT[:, m, a4 * P:(a4 + an) * P],
                                       pt[:, : an * P])
                    else:
                        nc.vector.tensor_copy(wcT[:, m, a4 * P:(a4 + an) * P],
                                              pt[:, : an * P])

            # ---- phase B: exchange boundary state ---------------------------
            cc_in = dram.tile([P, n_tiles], F32, addr_space="Local")
            cc_out = dram.tile([P, n_tiles], F32, addr_space="Local")
            s_m = small.tile([P, n_tiles, 1], F32)
            # mask: only first-half cores contribute their final state
            nc.vector.tensor_scalar_mul(s_m, h_sb[:, :, TH - 1:TH], m_in_sb)
            nc.sync.dma_start(out=cc_in, in_=s_m[:, :, 0])
            nc.gpsimd.collective_compute(
                "AllReduce", AOP.add, replica_groups=RG,
                ins=[cc_in.opt()], outs=[cc_out.opt()],
            )
            hin_raw = small.tile([P, n_tiles], F32)
            nc.sync.dma_start(out=hin_raw, in_=cc_out)
            hin = small.tile([P, n_tiles], F32)
            # only second-half cores apply the incoming state
            nc.vector.tensor_scalar_mul(hin, hin_raw, m_out_sb)

            # H = C * h_in + L, chunked so GEMM3 starts after the first chunk
            FIX = TH // 4
            for f in range(4):
                fs = slice(f * FIX, (f + 1) * FIX)
                for n in range(n_tiles):
                    nc.vector.scalar_tensor_tensor(
                        hf_sb[:, n, fs], c_sb[:, n, fs], hin[:, n:n + 1],
                        h_sb[:, n, fs], AOP.mult, AOP.add,
                    )

            # ---- phase C: GEMM3, back to natural layout ---------------------
            y_r = y.rearrange("(tt p) d -> tt p d", p=P)
            for tt in range(t_tiles):
                ps_ys = [
                    (psum_g if dc % 2 == 0 else psum_t).tile(
                        [P, DC], F32, name=f"py{dc}",
                        tag="psg" if dc % 2 == 0 else "pt",
                    )
                    for dc in range(d_chunks)
                ]
                for n in range(n_tiles):
                    lhsT = hf_sb[:, n, tt * P:(tt + 1) * P]
                    for dc in range(d_chunks):
                        nc.tensor.matmul(
                            ps_ys[dc], lhsT,
                            wcT[:, n, dc * DC:(dc + 1) * DC],
                            start=(n == 0), stop=(n == n_tiles - 1),
                        )
                y_t = y_pool.tile([P, D], F32, tag="yt", name=f"yt{tt}")
                for dc in range(d_chunks):
                    if dc % 2 == 0:
                        nc.scalar.copy(y_t[:, dc * DC:(dc + 1) * DC], ps_ys[dc])
                    else:
                        nc.vector.tensor_copy(y_t[:, dc * DC:(dc + 1) * DC],
                                              ps_ys[dc])
                nc.sync.dma_start(out=y_r[tt], in_=y_t)

    nc.compile()
    _module_cache[key] = nc
    return nc


def make_in_maps(u_full, Wl, bl, Wb, Wc, TH):
    """Per-core input dicts. Core c -> (batch c//2, half c%2)."""
    P = 128
    in_maps = []
    for c in range(N_CORES):
        b, half = c // 2, c % 2
        in_maps.append({
            "u": np.ascontiguousarray(u_full[b, half * TH:(half + 1) * TH, :]),
            "wl": Wl,
            "wb": Wb,
            "wc": Wc,
            "bl": bl,
            "m_in": np.full([P], 1.0 - half, np.float32),
            "m_out": np.full([P], float(half), np.float32),
        })
    return in_maps


def kernel(u, Wl, bl, Wb, Wc, Dp):
    global LAST_RESULTS
    u = np.asarray(u, np.float32)
    Wl = np.ascontiguousarray(np.asarray(Wl, np.float32))
    bl = np.ascontiguousarray(np.asarray(bl, np.float32))
    Wb = np.ascontiguousarray(np.asarray(Wb, np.float32))
    Wc = np.ascontiguousarray(np.asarray(Wc, np.float32))
    Dp = np.asarray(Dp, np.float32)

    B, T, D = u.shape
    N = Wl.shape[0]
    TH = T // 2
    nc = build_module(TH, D, N, 512)
    in_maps = make_in_maps(u, Wl, bl, Wb, Wc, TH)
    res = bass_utils.run_bass_kernel_spmd(
        nc, in_maps, core_ids=list(range(N_CORES))
    )
    LAST_RESULTS = res
    y = np.empty((B, T, D), np.float32)
    for c in range(N_CORES):
        b, half = c // 2, c % 2
        y[b, half * TH:(half + 1) * TH, :] = res.results[c]["y"]
    y += u * Dp[None, None, :]
    return y
